# revision 23
# baseline (speedup 1.0000x reference)
"""PointNet++ FeaturePropagation Trainium2 kernel (8-core SPMD).

Per core c of 8: batch b = c//2, query-half h = c%2 (8192 original points).
Software-pipelined over groups of gs tiles (128 queries each):
  1. PE computes KNN scores s(q,j) = 2 q.xyz_j - ||xyz_j||^2 (fp32, exact)
     against all 4096 sampled points; Act copies PSUM->SBUF rows.
  2. DVE max/max_index extract top-3 values + indices; inverse-distance
     weights batched per group.
  3. Indirect DMA gathers bf16 sampled-feature rows by on-chip indices.
  4. GpSimd computes the weighted interp; PE transpose to [C, q]; bf16
     pointwise MLP matmuls; BatchNorm stats ride Act accum_out and merge
     via 8-core AllGather (conv biases cancel through BN and are skipped);
     ReLU fused into BN apply on ACT.
Group g's gather/interp/mm0 instructions are emitted after group g+1's
score copies so the in-order Act/PE queues never stall the DVE scans.
Host does layout transforms (transpose/reshape/bf16 casts) and the final
unshard.
"""

import ml_dtypes
import numpy as np

import concourse.bass as bass
import concourse.bacc as bacc
import concourse.mybir as mybir
import concourse.tile as tile

F32 = mybir.dt.float32
BF16 = mybir.dt.bfloat16
U32 = mybir.dt.uint32
I16 = mybir.dt.int16
ALU = mybir.AluOpType
ACT = mybir.ActivationFunctionType
AX = mybir.AxisListType

B, S, N = 4, 4096, 16384
CS, CO = 256, 128
C1, C2 = 256, 256
NCORES = 8
QP = N // 2
BN_EPS = 1e-5
W_EPS = 1e-8


def build_program(n_t=QP // 128, gs=4, n_cores=NCORES, n_points_total=None):
    nq = n_t * 128
    nst = S // 128
    n_g = n_t // gs
    assert n_t % gs == 0
    if n_points_total is None:
        n_points_total = n_cores * nq
    nc = bacc.Bacc("TRN2", target_bir_lowering=False, debug=False,
                   num_devices=n_cores)

    d_xT4 = nc.dram_tensor("xT4", [32, nq], F32, kind="ExternalInput")
    d_rhsS = nc.dram_tensor("rhsS", [32, S], F32, kind="ExternalInput")
    d_xn2 = nc.dram_tensor("xn2", [128, n_t], F32, kind="ExternalInput")
    d_sfeatT = nc.dram_tensor("sfeatT", [S, CS], BF16, kind="ExternalInput")
    d_ofT = nc.dram_tensor("ofT", [CO, nq], BF16, kind="ExternalInput")
    d_w0T = nc.dram_tensor("w0T", [128, 3, C1], BF16, kind="ExternalInput")
    d_w1T = nc.dram_tensor("w1T", [128, 2, C2], BF16, kind="ExternalInput")
    d_g0 = nc.dram_tensor("g0p", [128, 2], F32, kind="ExternalInput")
    d_bt0 = nc.dram_tensor("bt0p", [128, 2], F32, kind="ExternalInput")
    d_g1 = nc.dram_tensor("g1p", [128, 2], F32, kind="ExternalInput")
    d_bt1 = nc.dram_tensor("bt1p", [128, 2], F32, kind="ExternalInput")
    d_eye = nc.dram_tensor("eye", [128, 128], F32, kind="ExternalInput")
    d_out = nc.dram_tensor("yT", [2, 128, nq], F32, kind="ExternalOutput")

    with tile.TileContext(nc) as tc:
        with (
            tc.tile_pool(name="const", bufs=1) as cpool,
            tc.tile_pool(name="big", bufs=1) as bigp,
            tc.tile_pool(name="sc_sb", bufs=2) as scp,
            tc.tile_pool(name="small", bufs=3) as smp,
            tc.tile_pool(name="grp", bufs=2) as grp,
            tc.tile_pool(name="outp", bufs=3) as outp,
            tc.tile_pool(name="ps_sc", bufs=2, space="PSUM") as ps_sc,
            tc.tile_pool(name="ps_sm", bufs=2, space="PSUM") as ps_sm,
            tc.tile_pool(name="ps_mm1", bufs=2, space="PSUM") as ps_mm1,
            tc.tile_pool(name="dram", bufs=1, space="DRAM") as dramp,
        ):
            # ---------------- resident loads ----------------
            def load(pool, name, dram, shape, dt=F32):
                t_ = pool.tile(shape, dt, tag=name)
                nc.sync.dma_start(t_[:], dram[:])
                return t_

            # ---------------- hot inputs first (startup latency) ----------
            rhsS = cpool.tile([32, S], F32, tag="rhsS")
            qtr = S // 4
            for i4 in range(4):
                eng = nc.scalar if i4 % 2 else nc.sync
                eng.dma_start(rhsS[:, i4 * qtr:(i4 + 1) * qtr],
                              d_rhsS[:, i4 * qtr:(i4 + 1) * qtr])
            xn2 = cpool.tile([128, n_t], F32, tag="xn2")
            nc.scalar.dma_start(xn2[:], d_xn2[:])
            xg_pre = {}

            def load_xg(t0, gsz):
                xg = smp.tile([32, gs * 128], F32, tag="xT4g")
                nc.sync.dma_start(
                    xg[:, 0:gsz * 128],
                    d_xT4[:, t0 * 128:(t0 + gsz) * 128])
                return xg

            xg_pre[0] = load_xg(0, gs)

            eye = load(cpool, "eye", d_eye, [128, 128])
            w0T = load(cpool, "w0T", d_w0T, [128, 3, C1], BF16)
            w1T = load(cpool, "w1T", d_w1T, [128, 2, C2], BF16)
            g0p = load(cpool, "g0p", d_g0, [128, 2])
            bt0p = load(cpool, "bt0p", d_bt0, [128, 2])
            g1p = load(cpool, "g1p", d_g1, [128, 2])
            bt1p = load(cpool, "bt1p", d_bt1, [128, 2])

            y0a = bigp.tile([128, nq], F32, tag="y0a")
            y0b = bigp.tile([128, nq], F32, tag="y0b")
            # BN stat partials: per (m, tile) accum columns
            s0p = cpool.tile([128, 2 * n_t], F32, tag="s0p")
            q0p = cpool.tile([128, 2 * n_t], F32, tag="q0p")
            scr128 = cpool.tile([128, 128], F32, tag="scr128")
            scr512 = cpool.tile([128, 512], F32, tag="scr512")

            # ---------------- per-group pipeline ----------------
            def knn_phase(t0, gsz):
                """PE scores + Act copies + DVE top-3 + weights for a group."""
                xg = xg_pre.pop(t0) if t0 in xg_pre else load_xg(t0, gsz)
                v8g_f = grp.tile([128, gs, 8], F32, tag="v8g")
                i8g_f = grp.tile([128, gs, 8], U32, tag="i8g")
                d3g_f = grp.tile([128, gs, 3], F32, tag="d3g")
                v8g = v8g_f[:, 0:gsz, :]
                i8g = i8g_f[:, 0:gsz, :]
                d3g = d3g_f[:, 0:gsz, :]
                for tt in range(gsz):
                    t = t0 + tt
                    sc = scp.tile([128, S], F32, tag="scores")
                    for q4 in range(4):
                        ps = ps_sc.tile([128, 1024], F32, tag="ps_score")
                        col = q4 * 1024
                        for half in range(2):
                            nc.tensor.matmul(
                                ps[:, half * 512:(half + 1) * 512],
                                xg[:, tt * 128:tt * 128 + 128],
                                rhsS[:, col + half * 512:col + (half + 1) * 512],
                                start=True, stop=True)
                        nc.scalar.copy(sc[:, col:col + 1024], ps[:])
                    nc.vector.max(v8g[:, tt, :], sc[:])
                    nc.vector.max_index(i8g[:, tt, :], v8g[:, tt, :], sc[:])
                    nc.vector.tensor_scalar(
                        out=d3g[:, tt, :], in0=v8g[:, tt, 0:3],
                        scalar1=xn2[:, t:t + 1], scalar2=-1.0,
                        op0=ALU.subtract, op1=ALU.mult)
                # batched weight math for the group
                nc.vector.tensor_scalar_add(d3g[:], d3g[:], W_EPS)
                r3_f = grp.tile([128, gs, 3], F32, tag="r3")
                r3 = r3_f[:, 0:gsz, :]
                nc.vector.reciprocal(r3, d3g[:])
                rs_f = grp.tile([128, gs], F32, tag="rs")
                nc.vector.tensor_reduce(out=rs_f[:, 0:gsz], in_=r3, axis=AX.X,
                                        op=ALU.add)
                rsr_f = grp.tile([128, gs], F32, tag="rsr")
                nc.vector.reciprocal(rsr_f[:, 0:gsz], rs_f[:, 0:gsz])
                wg = grp.tile([128, gs, 3], F32, tag="wg")
                for tt in range(gsz):
                    nc.vector.tensor_scalar(
                        out=wg[:, tt, :], in0=r3_f[:, tt, :],
                        scalar1=rsr_f[:, tt:tt + 1], scalar2=None, op0=ALU.mult)
                # gather indices: int16, wrapped in 16 partitions
                idx16 = grp.tile([128, gs, 3], I16, tag="idx16")
                nc.vector.tensor_copy(idx16[:, 0:gsz, :], i8g[:, :, 0:3])
                wrapped = grp.tile([128, gs * 24], I16, tag="wrapped")
                ncols = gsz * 24
                for u in range(8):
                    nc.gpsimd.dma_start(
                        wrapped[0:16, u:ncols:8].rearrange(
                            "p (t k) -> p t k", k=3),
                        idx16[16 * u:16 * (u + 1), 0:gsz, :])
                for rep in range(1, 8):
                    nc.gpsimd.dma_start(wrapped[16 * rep:16 * (rep + 1), 0:ncols],
                                        wrapped[0:16, 0:ncols])
                gbuf = grp.tile([128, gs * 3, CS], BF16, tag="gath")
                nc.gpsimd.dma_gather(
                    out_ap=gbuf[:, 0:gsz * 3, :],
                    in_ap=d_sfeatT[:],
                    idxs_ap=wrapped[:, 0:ncols],
                    num_idxs=gsz * 384,
                    num_idxs_reg=gsz * 384,
                    elem_size=CS,
                )
                ofTg = grp.tile([CO, gs * 128], BF16, tag="ofTg")
                nc.sync.dma_start(ofTg[:, 0:gsz * 128],
                                  d_ofT[:, t0 * 128:(t0 + gsz) * 128])
                return wg, gbuf, ofTg

            def interp_mm0_phase(t0, gsz, wg, gbuf, ofTg):
                """GpSimd interp + PE transpose/mm0 + Act y0 copies."""
                for tt in range(gsz):
                    t = t0 + tt
                    interp = grp.tile([128, CS], F32, tag="interp")
                    nc.gpsimd.scalar_tensor_tensor(
                        out=interp[:], in0=gbuf[:, tt * 3, :],
                        scalar=wg[:, tt, 0:1], in1=gbuf[:, tt * 3, :],
                        op0=ALU.mult, op1=ALU.bypass)
                    for k in (1, 2):
                        nc.gpsimd.scalar_tensor_tensor(
                            out=interp[:], in0=gbuf[:, tt * 3 + k, :],
                            scalar=wg[:, tt, k:k + 1], in1=interp[:],
                            op0=ALU.mult, op1=ALU.add)
                    iT = grp.tile([128, 2, 128], BF16, tag="interpT")
                    for hh in range(2):
                        ps_tr = ps_sm.tile([128, 128], F32, tag="ps_small")
                        nc.tensor.transpose(
                            ps_tr[:], interp[:, hh * 128:(hh + 1) * 128],
                            eye[:])
                        nc.scalar.activation(out=iT[:, hh, :], in_=ps_tr[:],
                                             func=ACT.Copy)
                    for m, ybuf in ((0, y0a), (1, y0b)):
                        ps_y = ps_sm.tile([128, 128], F32, tag="ps_small")
                        mcol = slice(m * 128, (m + 1) * 128)
                        nc.tensor.matmul(ps_y[:], w0T[:, 0, mcol],
                                         ofTg[:, tt * 128:(tt + 1) * 128],
                                         start=True, stop=False)
                        nc.tensor.matmul(ps_y[:], w0T[:, 1, mcol],
                                         iT[:, 0, :], start=False, stop=False)
                        nc.tensor.matmul(ps_y[:], w0T[:, 2, mcol],
                                         iT[:, 1, :], start=False, stop=True)
                        col = m * n_t + t
                        nc.scalar.activation(
                            out=ybuf[:, t * 128:(t + 1) * 128], in_=ps_y[:],
                            func=ACT.Copy, accum_out=s0p[:, col:col + 1])
                        nc.scalar.activation(
                            out=scr128[:], in_=ps_y[:], func=ACT.Square,
                            accum_out=q0p[:, col:col + 1])

            # ---------------- BN via AllGather ----------------
            def gather_stats(sp, qp_, tag, lo, hi):
                """AllGather the per-(m,tile) partial sums over cols [lo,hi)."""
                stats = cpool.tile([128, 4], F32, tag=f"stats{tag}")
                nc.vector.tensor_reduce(
                    out=stats[:, 0:2],
                    in_=sp.rearrange("p (m c) -> p m c", m=2)[:, :, lo:hi],
                    axis=AX.X, op=ALU.add)
                nc.vector.tensor_reduce(
                    out=stats[:, 2:4],
                    in_=qp_.rearrange("p (m c) -> p m c", m=2)[:, :, lo:hi],
                    axis=AX.X, op=ALU.add)
                bi = dramp.tile([128, 4], F32, tag=f"bi{tag}")
                bo = dramp.tile([n_cores, 128, 4], F32, tag=f"bo{tag}")
                nc.gpsimd.dma_start(bi[:], stats[:])
                nc.gpsimd.collective_compute(
                    "AllGather", ALU.bypass,
                    replica_groups=[list(range(n_cores))],
                    ins=[bi.opt()], outs=[bo.opt()])
                # DRAM AllGather output is replica-major flat: load as
                # [n_cores, 512], reduce across partitions, scatter back.
                gath = cpool.tile([n_cores, 512], F32, tag=f"gath{tag}")
                nc.gpsimd.dma_start(
                    gath[:], bo[:].rearrange("r p s -> r (p s)"))
                gsum = cpool.tile([1, 512], F32, tag=f"gsum{tag}")
                nc.gpsimd.tensor_reduce(out=gsum[:], in_=gath[:],
                                        axis=AX.C, op=ALU.add)
                gstats = cpool.tile([128, 4], F32, tag=f"gstats{tag}")
                nc.gpsimd.dma_start(
                    gstats[:], gsum[:].rearrange("r (p s) -> r p s", p=128))
                return gstats

            def bn_stats(gp, btp, tag, gstats_parts):
                """Combine gathered stats -> (a, bhat); yhat=Relu(y*a+bhat)."""
                if len(gstats_parts) == 1:
                    gstats = gstats_parts[0]
                else:
                    gstats = cpool.tile([128, 4], F32, tag=f"gsts{tag}")
                    nc.vector.tensor_tensor(
                        out=gstats[:], in0=gstats_parts[0][:],
                        in1=gstats_parts[1][:], op=ALU.add)

                mean = cpool.tile([128, 2], F32, tag=f"mean{tag}")
                nc.vector.tensor_scalar_mul(mean[:], gstats[:, 0:2],
                                            1.0 / n_points_total)
                vpe = cpool.tile([128, 2], F32, tag=f"vpe{tag}")
                nc.vector.tensor_scalar_mul(vpe[:], gstats[:, 2:4],
                                            1.0 / n_points_total)
                msq = cpool.tile([128, 2], F32, tag=f"msq{tag}")
                nc.vector.tensor_tensor(out=msq[:], in0=mean[:], in1=mean[:],
                                        op=ALU.mult)
                nc.vector.tensor_tensor(out=vpe[:], in0=vpe[:], in1=msq[:],
                                        op=ALU.subtract)
                nc.vector.tensor_scalar_add(vpe[:], vpe[:], BN_EPS)
                rcp = cpool.tile([128, 2], F32, tag=f"rcp{tag}")
                nc.vector.reciprocal(rcp[:], vpe[:])
                rsq = cpool.tile([128, 2], F32, tag=f"rsq{tag}")
                nc.scalar.activation(out=rsq[:], in_=rcp[:], func=ACT.Sqrt)
                t1 = cpool.tile([128, 2], F32, tag=f"t1{tag}")
                nc.vector.tensor_tensor(out=t1[:], in0=rsq[:], in1=rsq[:],
                                        op=ALU.mult)
                nc.vector.tensor_tensor(out=t1[:], in0=t1[:], in1=vpe[:],
                                        op=ALU.mult)
                nc.vector.tensor_scalar(out=t1[:], in0=t1[:], scalar1=-0.5,
                                        scalar2=1.5, op0=ALU.mult, op1=ALU.add)
                nc.vector.tensor_tensor(out=rsq[:], in0=rsq[:], in1=t1[:],
                                        op=ALU.mult)
                a = cpool.tile([128, 2], F32, tag=f"a{tag}")
                nc.vector.tensor_tensor(out=a[:], in0=gp[:], in1=rsq[:],
                                        op=ALU.mult)
                bhat = cpool.tile([128, 2], F32, tag=f"bhat{tag}")
                nc.vector.tensor_tensor(out=bhat[:], in0=mean[:], in1=a[:],
                                        op=ALU.mult)
                nc.vector.tensor_tensor(out=bhat[:], in0=btp[:], in1=bhat[:],
                                        op=ALU.subtract)
                return a, bhat

            # groups of gs tiles; split the final group in half so the
            # post-scan drain (gather/interp/mm0 of the last group) is short
            gsched = []
            t0 = 0
            while t0 < n_t:
                rem = n_t - t0
                if rem > gs or rem <= 2 or gs < 4:
                    gsz = min(gs, rem)
                else:
                    gsz = rem // 2
                gsched.append((t0, gsz))
                t0 += gsz
            t_cut = gsched[-3][0] if len(gsched) >= 3 else 0
            g0A = None
            prev = None
            for (tg, gsz) in gsched:
                cur = knn_phase(tg, gsz)
                if prev is not None:
                    interp_mm0_phase(*prev)
                    if prev[0] + prev[1] == t_cut and t_cut > 0:
                        g0A = gather_stats(s0p[:], q0p[:], "0A", 0, t_cut)
                prev = (tg, gsz, *cur)
            interp_mm0_phase(*prev)

            # ---------------- layer 1 ----------------
            g0B = gather_stats(s0p[:], q0p[:], "0B", t_cut, n_t)
            a0, b0h = bn_stats(g0p, bt0p, "0",
                               [g0A, g0B] if g0A is not None else [g0B])
            csz = min(512, nq)
            nchunk = nq // csz
            ch_cut = max(0, nchunk - 3)
            g1A = None
            s1p = cpool.tile([128, 2 * nchunk], F32, tag="s1p")
            q1p = cpool.tile([128, 2 * nchunk], F32, tag="q1p")
            for ch in range(nchunk):
                sl = slice(ch * csz, (ch + 1) * csz)
                yh0 = smp.tile([128, csz], BF16, tag="yh0")
                yh1 = smp.tile([128, csz], BF16, tag="yh1")
                nc.scalar.activation(out=yh0[:], in_=y0a[:, sl], func=ACT.Relu,
                                     scale=a0[:, 0:1], bias=b0h[:, 0:1])
                nc.scalar.activation(out=yh1[:], in_=y0b[:, sl], func=ACT.Relu,
                                     scale=a0[:, 1:2], bias=b0h[:, 1:2])
                for m, ybuf in ((0, y0a), (1, y0b)):
                    ps1 = ps_mm1.tile([128, csz], F32, tag="ps_mm1")
                    mcol = slice(m * 128, (m + 1) * 128)
                    nc.tensor.matmul(ps1[:], w1T[:, 0, mcol], yh0[:],
                                     start=True, stop=False)
                    nc.tensor.matmul(ps1[:], w1T[:, 1, mcol], yh1[:],
                                     start=False, stop=True)
                    col = m * nchunk + ch
                    if m == 0:
                        # Act: copy + sum accum
                        nc.scalar.activation(
                            out=ybuf[:, sl], in_=ps1[:], func=ACT.Copy,
                            accum_out=s1p[:, col:col + 1])
                    else:
                        # DVE: copy + sum accum in one stt
                        nc.vector.scalar_tensor_tensor(
                            out=ybuf[:, sl], in0=ps1[:], scalar=0.0,
                            in1=ps1[:], op0=ALU.bypass, op1=ALU.bypass,
                            accum_out=s1p[:, col:col + 1])
                    nc.vector.scalar_tensor_tensor(
                        out=scr512[:, 0:csz], in0=ybuf[:, sl], scalar=0.0,
                        in1=ybuf[:, sl], op0=ALU.bypass, op1=ALU.mult,
                        accum_out=q1p[:, col:col + 1])
                if ch + 1 == ch_cut and ch_cut > 0:
                    g1A = gather_stats(s1p[:], q1p[:], "1A", 0, ch_cut)

            # ---------------- layer 2 BN + out ----------------
            g1B = gather_stats(s1p[:], q1p[:], "1B", ch_cut, nchunk)
            a1, b1h = bn_stats(g1p, bt1p, "1",
                               [g1A, g1B] if g1A is not None else [g1B])
            for ch in range(nchunk):
                sl = slice(ch * csz, (ch + 1) * csz)
                o = outp.tile([128, csz], F32, tag="outsb")
                nc.scalar.activation(out=o[:], in_=y0a[:, sl],
                                     func=ACT.Relu,
                                     scale=a1[:, 0:1],
                                     bias=b1h[:, 0:1])
                nc.sync.dma_start(d_out[0, :, sl], o[:])
                o2 = outp.tile([128, csz], F32, tag="outsb2")
                nc.vector.tensor_scalar(out=o2[:], in0=y0b[:, sl],
                                        scalar1=a1[:, 1:2],
                                        scalar2=b1h[:, 1:2],
                                        op0=ALU.mult, op1=ALU.add)
                nc.vector.tensor_scalar_max(o2[:], o2[:], 0.0)
                nc.scalar.dma_start(d_out[1, :, sl], o2[:])

    nc.compile()
    return nc


def make_core_inputs(sampled_xyz, sampled_features, original_xyz,
                     original_features, w0, w1, g0, bt0, g1, bt1,
                     core, n_t=QP // 128):
    """Host-side layout prep for one core (transposes/reshapes/casts only)."""
    nq = n_t * 128
    nst = S // 128
    b, h = core // 2, core % 2
    ox = original_xyz[b, h * nq:(h + 1) * nq]          # [nq, 3]
    of = original_features[b, h * nq:(h + 1) * nq]     # [nq, CO]
    sx = sampled_xyz[b]                                # [S, 3]
    f32 = np.float32
    bf16 = ml_dtypes.bfloat16
    xT4 = np.concatenate([ox.T, np.ones((1, nq), f32),
                          np.zeros((28, nq), f32)], 0).astype(f32)
    sn2 = (sx.astype(f32) * sx.astype(f32)).sum(-1).astype(f32)
    rhsS = np.concatenate([(f32(2) * sx.T).astype(f32), -sn2[None, :],
                           np.zeros((28, S), f32)], 0).astype(f32)
    xn2 = (ox.astype(f32) * ox.astype(f32)).sum(-1).astype(f32)
    xn2 = np.ascontiguousarray(xn2.reshape(n_t, 128).T).astype(f32)
    return {
        "xT4": xT4,
        "rhsS": rhsS,
        "xn2": xn2,
        "sfeatT": np.ascontiguousarray(sampled_features[b]).astype(bf16),
        "ofT": np.ascontiguousarray(of.T).astype(bf16),
        "w0T": np.ascontiguousarray(
            w0.T.reshape(3, 128, C1).transpose(1, 0, 2)).astype(bf16),
        "w1T": np.ascontiguousarray(
            w1.T.reshape(2, 128, C2).transpose(1, 0, 2)).astype(bf16),
        "g0p": np.ascontiguousarray(g0.reshape(2, 128).T).astype(f32),
        "bt0p": np.ascontiguousarray(bt0.reshape(2, 128).T).astype(f32),
        "g1p": np.ascontiguousarray(g1.reshape(2, 128).T).astype(f32),
        "bt1p": np.ascontiguousarray(bt1.reshape(2, 128).T).astype(f32),
        "eye": np.eye(128, dtype=f32),
    }


_PROGRAM_CACHE = {}


def kernel(sampled_xyz, sampled_features, original_xyz, original_features,
           w0, b0, g0, bt0, w1, b1, g1, bt1, k):
    assert int(k) == 3
    from concourse.bass_utils import run_bass_kernel_spmd

    key = "full"
    if key not in _PROGRAM_CACHE:
        _PROGRAM_CACHE[key] = build_program()
    nc = _PROGRAM_CACHE[key]

    args = (sampled_xyz, sampled_features, original_xyz, original_features,
            w0, w1, g0, bt0, g1, bt1)
    in_maps = [make_core_inputs(*[np.asarray(a, np.float32) for a in args],
                                core=c) for c in range(NCORES)]
    res = run_bass_kernel_spmd(nc, in_maps, core_ids=list(range(NCORES)))
    out = np.empty((B, N, C2), np.float32)
    nq = QP
    for c in range(NCORES):
        b, h = c // 2, c % 2
        yT = res.results[c]["yT"]            # [2, 128, nq]
        y = yT.reshape(256, nq).T            # [nq, 256]
        out[b, h * nq:(h + 1) * nq] = y
    return out


# revision 24
# speedup vs baseline: 1.0299x; 1.0299x over previous
"""PointNet++ FeaturePropagation Trainium2 kernel (8-core SPMD).

Per core c of 8: batch b = c//2, query-half h = c%2 (8192 original points).
Software-pipelined over groups of gs tiles (128 queries each):
  1. PE computes KNN scores s(q,j) = 2 q.xyz_j - ||xyz_j||^2 (fp32, exact)
     against all 4096 sampled points; Act copies PSUM->SBUF rows.
  2. DVE max/max_index extract top-3 values + indices; inverse-distance
     weights batched per group.
  3. Indirect DMA gathers bf16 sampled-feature rows by on-chip indices.
  4. GpSimd computes the weighted interp; PE transpose to [C, q]; bf16
     pointwise MLP matmuls; BatchNorm stats ride Act accum_out and merge
     via 8-core AllGather (conv biases cancel through BN and are skipped);
     ReLU fused into BN apply on ACT.
Group g's gather/interp/mm0 instructions are emitted after group g+1's
score copies so the in-order Act/PE queues never stall the DVE scans.
Host does layout transforms (transpose/reshape/bf16 casts) and the final
unshard.
"""

import ml_dtypes
import numpy as np

import concourse.bass as bass
import concourse.bacc as bacc
import concourse.mybir as mybir
import concourse.tile as tile

F32 = mybir.dt.float32
BF16 = mybir.dt.bfloat16
U32 = mybir.dt.uint32
I16 = mybir.dt.int16
ALU = mybir.AluOpType
ACT = mybir.ActivationFunctionType
AX = mybir.AxisListType

B, S, N = 4, 4096, 16384
CS, CO = 256, 128
C1, C2 = 256, 256
NCORES = 8
QP = N // 2
BN_EPS = 1e-5
W_EPS = 1e-8


def build_program(n_t=QP // 128, gs=4, n_cores=NCORES, n_points_total=None):
    nq = n_t * 128
    nst = S // 128
    n_g = n_t // gs
    assert n_t % gs == 0
    if n_points_total is None:
        n_points_total = n_cores * nq
    nc = bacc.Bacc("TRN2", target_bir_lowering=False, debug=False,
                   num_devices=n_cores)

    d_xT4 = nc.dram_tensor("xT4", [32, nq], F32, kind="ExternalInput")
    d_rhsS = nc.dram_tensor("rhsS", [32, S], F32, kind="ExternalInput")
    d_xn2 = nc.dram_tensor("xn2", [128, n_t], F32, kind="ExternalInput")
    d_sfeatT = nc.dram_tensor("sfeatT", [S, CS], BF16, kind="ExternalInput")
    d_ofT = nc.dram_tensor("ofT", [CO, nq], BF16, kind="ExternalInput")
    d_w0T = nc.dram_tensor("w0T", [128, 3, C1], BF16, kind="ExternalInput")
    d_w1T = nc.dram_tensor("w1T", [128, 2, C2], BF16, kind="ExternalInput")
    d_g0 = nc.dram_tensor("g0p", [128, 2], F32, kind="ExternalInput")
    d_bt0 = nc.dram_tensor("bt0p", [128, 2], F32, kind="ExternalInput")
    d_g1 = nc.dram_tensor("g1p", [128, 2], F32, kind="ExternalInput")
    d_bt1 = nc.dram_tensor("bt1p", [128, 2], F32, kind="ExternalInput")
    d_eye = nc.dram_tensor("eye", [128, 128], F32, kind="ExternalInput")
    d_out = nc.dram_tensor("yT", [2, 128, nq], F32, kind="ExternalOutput")

    with tile.TileContext(nc) as tc:
        with (
            tc.tile_pool(name="const", bufs=1) as cpool,
            tc.tile_pool(name="big", bufs=1) as bigp,
            tc.tile_pool(name="sc_sb", bufs=2) as scp,
            tc.tile_pool(name="small", bufs=3) as smp,
            tc.tile_pool(name="grp", bufs=2) as grp,
            tc.tile_pool(name="outp", bufs=3) as outp,
            tc.tile_pool(name="ps_sc", bufs=2, space="PSUM") as ps_sc,
            tc.tile_pool(name="ps_sm", bufs=2, space="PSUM") as ps_sm,
            tc.tile_pool(name="ps_mm1", bufs=2, space="PSUM") as ps_mm1,
            tc.tile_pool(name="dram", bufs=1, space="DRAM") as dramp,
        ):
            # ---------------- resident loads ----------------
            def load(pool, name, dram, shape, dt=F32):
                t_ = pool.tile(shape, dt, tag=name)
                nc.sync.dma_start(t_[:], dram[:])
                return t_

            # ---------------- hot inputs first (startup latency) ----------
            rhsS = cpool.tile([32, S], F32, tag="rhsS")
            qtr = S // 4
            for i4 in range(4):
                eng = nc.scalar if i4 % 2 else nc.sync
                eng.dma_start(rhsS[:, i4 * qtr:(i4 + 1) * qtr],
                              d_rhsS[:, i4 * qtr:(i4 + 1) * qtr])
            xn2 = cpool.tile([128, n_t], F32, tag="xn2")
            nc.scalar.dma_start(xn2[:], d_xn2[:])
            xg_pre = {}

            def load_xg(t0, gsz):
                xg = smp.tile([32, gs * 128], F32, tag="xT4g")
                nc.sync.dma_start(
                    xg[:, 0:gsz * 128],
                    d_xT4[:, t0 * 128:(t0 + gsz) * 128])
                return xg

            xg_pre[0] = load_xg(0, gs)

            eye = load(cpool, "eye", d_eye, [128, 128])
            w0T = load(cpool, "w0T", d_w0T, [128, 3, C1], BF16)
            w1T = load(cpool, "w1T", d_w1T, [128, 2, C2], BF16)
            g0p = load(cpool, "g0p", d_g0, [128, 2])
            bt0p = load(cpool, "bt0p", d_bt0, [128, 2])
            g1p = load(cpool, "g1p", d_g1, [128, 2])
            bt1p = load(cpool, "bt1p", d_bt1, [128, 2])

            y0a = bigp.tile([128, nq], F32, tag="y0a")
            y0b = bigp.tile([128, nq], F32, tag="y0b")
            # BN stat partials: per (m, tile) accum columns
            s0p = cpool.tile([128, 2 * n_t], F32, tag="s0p")
            q0p = cpool.tile([128, 2 * n_t], F32, tag="q0p")
            scr128 = cpool.tile([128, 128], F32, tag="scr128")
            scr512 = cpool.tile([128, 512], F32, tag="scr512")

            # ---------------- per-group pipeline ----------------
            def knn_phase(t0, gsz):
                """PE scores + Act copies + DVE top-3 + weights for a group."""
                xg = xg_pre.pop(t0) if t0 in xg_pre else load_xg(t0, gsz)
                v8g_f = grp.tile([128, gs, 8], F32, tag="v8g")
                i8g_f = grp.tile([128, gs, 8], U32, tag="i8g")
                d3g_f = grp.tile([128, gs, 3], F32, tag="d3g")
                v8g = v8g_f[:, 0:gsz, :]
                i8g = i8g_f[:, 0:gsz, :]
                d3g = d3g_f[:, 0:gsz, :]
                for tt in range(gsz):
                    t = t0 + tt
                    sc = scp.tile([128, S], F32, tag="scores")
                    for q4 in range(4):
                        ps = ps_sc.tile([128, 1024], F32, tag="ps_score")
                        col = q4 * 1024
                        for half in range(2):
                            nc.tensor.matmul(
                                ps[:, half * 512:(half + 1) * 512],
                                xg[:, tt * 128:tt * 128 + 128],
                                rhsS[:, col + half * 512:col + (half + 1) * 512],
                                start=True, stop=True)
                        nc.scalar.copy(sc[:, col:col + 1024], ps[:])
                    nc.vector.max(v8g[:, tt, :], sc[:])
                    nc.vector.max_index(i8g[:, tt, :], v8g[:, tt, :], sc[:])
                    nc.vector.tensor_scalar(
                        out=d3g[:, tt, :], in0=v8g[:, tt, 0:3],
                        scalar1=xn2[:, t:t + 1], scalar2=-1.0,
                        op0=ALU.subtract, op1=ALU.mult)
                # batched weight math for the group
                nc.vector.tensor_scalar_add(d3g[:], d3g[:], W_EPS)
                r3_f = grp.tile([128, gs, 3], F32, tag="r3")
                r3 = r3_f[:, 0:gsz, :]
                nc.vector.reciprocal(r3, d3g[:])
                rs_f = grp.tile([128, gs], F32, tag="rs")
                nc.vector.tensor_reduce(out=rs_f[:, 0:gsz], in_=r3, axis=AX.X,
                                        op=ALU.add)
                rsr_f = grp.tile([128, gs], F32, tag="rsr")
                nc.vector.reciprocal(rsr_f[:, 0:gsz], rs_f[:, 0:gsz])
                wg = grp.tile([128, gs, 3], F32, tag="wg")
                for tt in range(gsz):
                    nc.vector.tensor_scalar(
                        out=wg[:, tt, :], in0=r3_f[:, tt, :],
                        scalar1=rsr_f[:, tt:tt + 1], scalar2=None, op0=ALU.mult)
                # gather indices: int16, wrapped in 16 partitions
                idx16 = grp.tile([128, gs, 3], I16, tag="idx16")
                nc.vector.tensor_copy(idx16[:, 0:gsz, :], i8g[:, :, 0:3])
                wrapped = grp.tile([128, gs * 24], I16, tag="wrapped")
                ncols = gsz * 24
                for u in range(8):
                    nc.gpsimd.dma_start(
                        wrapped[0:16, u:ncols:8].rearrange(
                            "p (t k) -> p t k", k=3),
                        idx16[16 * u:16 * (u + 1), 0:gsz, :])
                for rep in range(1, 8):
                    nc.gpsimd.dma_start(wrapped[16 * rep:16 * (rep + 1), 0:ncols],
                                        wrapped[0:16, 0:ncols])
                gbuf = grp.tile([128, gs * 3, CS], BF16, tag="gath")
                nc.gpsimd.dma_gather(
                    out_ap=gbuf[:, 0:gsz * 3, :],
                    in_ap=d_sfeatT[:],
                    idxs_ap=wrapped[:, 0:ncols],
                    num_idxs=gsz * 384,
                    num_idxs_reg=gsz * 384,
                    elem_size=CS,
                )
                ofTg = grp.tile([CO, gs * 128], BF16, tag="ofTg")
                nc.sync.dma_start(ofTg[:, 0:gsz * 128],
                                  d_ofT[:, t0 * 128:(t0 + gsz) * 128])
                return wg, gbuf, ofTg

            def interp_mm0_phase(t0, gsz, wg, gbuf, ofTg):
                """GpSimd interp + PE transpose/mm0 + Act y0 copies."""
                for tt in range(gsz):
                    t = t0 + tt
                    interp = grp.tile([128, CS], F32, tag="interp")
                    nc.gpsimd.scalar_tensor_tensor(
                        out=interp[:], in0=gbuf[:, tt * 3, :],
                        scalar=wg[:, tt, 0:1], in1=gbuf[:, tt * 3, :],
                        op0=ALU.mult, op1=ALU.bypass)
                    for k in (1, 2):
                        nc.gpsimd.scalar_tensor_tensor(
                            out=interp[:], in0=gbuf[:, tt * 3 + k, :],
                            scalar=wg[:, tt, k:k + 1], in1=interp[:],
                            op0=ALU.mult, op1=ALU.add)
                    iT = grp.tile([128, 2, 128], BF16, tag="interpT")
                    for hh in range(2):
                        ps_tr = ps_sm.tile([128, 128], F32, tag="ps_small")
                        nc.tensor.transpose(
                            ps_tr[:], interp[:, hh * 128:(hh + 1) * 128],
                            eye[:])
                        nc.scalar.activation(out=iT[:, hh, :], in_=ps_tr[:],
                                             func=ACT.Copy)
                    for m, ybuf in ((0, y0a), (1, y0b)):
                        ps_y = ps_sm.tile([128, 128], F32, tag="ps_small")
                        mcol = slice(m * 128, (m + 1) * 128)
                        nc.tensor.matmul(ps_y[:], w0T[:, 0, mcol],
                                         ofTg[:, tt * 128:(tt + 1) * 128],
                                         start=True, stop=False)
                        nc.tensor.matmul(ps_y[:], w0T[:, 1, mcol],
                                         iT[:, 0, :], start=False, stop=False)
                        nc.tensor.matmul(ps_y[:], w0T[:, 2, mcol],
                                         iT[:, 1, :], start=False, stop=True)
                        col = m * n_t + t
                        nc.scalar.activation(
                            out=ybuf[:, t * 128:(t + 1) * 128], in_=ps_y[:],
                            func=ACT.Copy, accum_out=s0p[:, col:col + 1])
                        nc.scalar.activation(
                            out=scr128[:], in_=ps_y[:], func=ACT.Square,
                            accum_out=q0p[:, col:col + 1])

            # ---------------- BN via AllGather ----------------
            def gather_stats(sp, qp_, tag, lo, hi):
                """AllGather the per-(m,tile) partial sums over cols [lo,hi)."""
                stats = cpool.tile([128, 4], F32, tag=f"stats{tag}")
                nc.vector.tensor_reduce(
                    out=stats[:, 0:2],
                    in_=sp.rearrange("p (m c) -> p m c", m=2)[:, :, lo:hi],
                    axis=AX.X, op=ALU.add)
                nc.vector.tensor_reduce(
                    out=stats[:, 2:4],
                    in_=qp_.rearrange("p (m c) -> p m c", m=2)[:, :, lo:hi],
                    axis=AX.X, op=ALU.add)
                bi = dramp.tile([128, 4], F32, tag=f"bi{tag}")
                bo = dramp.tile([n_cores, 128, 4], F32, tag=f"bo{tag}")
                nc.gpsimd.dma_start(bi[:], stats[:])
                nc.gpsimd.collective_compute(
                    "AllGather", ALU.bypass,
                    replica_groups=[list(range(n_cores))],
                    ins=[bi.opt()], outs=[bo.opt()])
                # DRAM AllGather output is replica-major flat: load as
                # [n_cores, 512], reduce across partitions, scatter back.
                gath = cpool.tile([n_cores, 512], F32, tag=f"gath{tag}")
                nc.gpsimd.dma_start(
                    gath[:], bo[:].rearrange("r p s -> r (p s)"))
                gsum = cpool.tile([1, 512], F32, tag=f"gsum{tag}")
                nc.gpsimd.tensor_reduce(out=gsum[:], in_=gath[:],
                                        axis=AX.C, op=ALU.add)
                gstats = cpool.tile([128, 4], F32, tag=f"gstats{tag}")
                nc.gpsimd.dma_start(
                    gstats[:], gsum[:].rearrange("r (p s) -> r p s", p=128))
                return gstats

            def bn_stats(gp, btp, tag, gstats_parts):
                """Combine gathered stats -> (a, bhat); yhat=Relu(y*a+bhat)."""
                if len(gstats_parts) == 1:
                    gstats = gstats_parts[0]
                else:
                    gstats = cpool.tile([128, 4], F32, tag=f"gsts{tag}")
                    nc.vector.tensor_tensor(
                        out=gstats[:], in0=gstats_parts[0][:],
                        in1=gstats_parts[1][:], op=ALU.add)

                mean = cpool.tile([128, 2], F32, tag=f"mean{tag}")
                nc.vector.tensor_scalar_mul(mean[:], gstats[:, 0:2],
                                            1.0 / n_points_total)
                vpe = cpool.tile([128, 2], F32, tag=f"vpe{tag}")
                nc.vector.tensor_scalar_mul(vpe[:], gstats[:, 2:4],
                                            1.0 / n_points_total)
                msq = cpool.tile([128, 2], F32, tag=f"msq{tag}")
                nc.vector.tensor_tensor(out=msq[:], in0=mean[:], in1=mean[:],
                                        op=ALU.mult)
                nc.vector.tensor_tensor(out=vpe[:], in0=vpe[:], in1=msq[:],
                                        op=ALU.subtract)
                nc.vector.tensor_scalar_add(vpe[:], vpe[:], BN_EPS)
                rcp = cpool.tile([128, 2], F32, tag=f"rcp{tag}")
                nc.vector.reciprocal(rcp[:], vpe[:])
                rsq = cpool.tile([128, 2], F32, tag=f"rsq{tag}")
                nc.scalar.activation(out=rsq[:], in_=rcp[:], func=ACT.Sqrt)
                t1 = cpool.tile([128, 2], F32, tag=f"t1{tag}")
                nc.vector.tensor_tensor(out=t1[:], in0=rsq[:], in1=rsq[:],
                                        op=ALU.mult)
                nc.vector.tensor_tensor(out=t1[:], in0=t1[:], in1=vpe[:],
                                        op=ALU.mult)
                nc.vector.tensor_scalar(out=t1[:], in0=t1[:], scalar1=-0.5,
                                        scalar2=1.5, op0=ALU.mult, op1=ALU.add)
                nc.vector.tensor_tensor(out=rsq[:], in0=rsq[:], in1=t1[:],
                                        op=ALU.mult)
                a = cpool.tile([128, 2], F32, tag=f"a{tag}")
                nc.vector.tensor_tensor(out=a[:], in0=gp[:], in1=rsq[:],
                                        op=ALU.mult)
                bhat = cpool.tile([128, 2], F32, tag=f"bhat{tag}")
                nc.vector.tensor_tensor(out=bhat[:], in0=mean[:], in1=a[:],
                                        op=ALU.mult)
                nc.vector.tensor_tensor(out=bhat[:], in0=btp[:], in1=bhat[:],
                                        op=ALU.subtract)
                return a, bhat

            # groups of gs tiles; split the final group in half so the
            # post-scan drain (gather/interp/mm0 of the last group) is short
            gsched = []
            t0 = 0
            while t0 < n_t:
                rem = n_t - t0
                if rem > gs or rem <= 2 or gs < 4:
                    gsz = min(gs, rem)
                else:
                    gsz = rem // 2
                gsched.append((t0, gsz))
                t0 += gsz
            prev = None
            for (tg, gsz) in gsched:
                cur = knn_phase(tg, gsz)
                if prev is not None:
                    interp_mm0_phase(*prev)
                prev = (tg, gsz, *cur)
            interp_mm0_phase(*prev)

            # ---------------- layer 1 ----------------
            g0 = gather_stats(s0p[:], q0p[:], "0", 0, n_t)
            a0, b0h = bn_stats(g0p, bt0p, "0", [g0])
            csz = min(512, nq)
            nchunk = nq // csz
            s1p = cpool.tile([128, 2 * nchunk], F32, tag="s1p")
            q1p = cpool.tile([128, 2 * nchunk], F32, tag="q1p")
            for ch in range(nchunk):
                sl = slice(ch * csz, (ch + 1) * csz)
                yh0 = smp.tile([128, csz], BF16, tag="yh0")
                yh1 = smp.tile([128, csz], BF16, tag="yh1")
                nc.scalar.activation(out=yh0[:], in_=y0a[:, sl], func=ACT.Relu,
                                     scale=a0[:, 0:1], bias=b0h[:, 0:1])
                nc.scalar.activation(out=yh1[:], in_=y0b[:, sl], func=ACT.Relu,
                                     scale=a0[:, 1:2], bias=b0h[:, 1:2])
                for m, ybuf in ((0, y0a), (1, y0b)):
                    ps1 = ps_mm1.tile([128, csz], F32, tag="ps_mm1")
                    mcol = slice(m * 128, (m + 1) * 128)
                    nc.tensor.matmul(ps1[:], w1T[:, 0, mcol], yh0[:],
                                     start=True, stop=False)
                    nc.tensor.matmul(ps1[:], w1T[:, 1, mcol], yh1[:],
                                     start=False, stop=True)
                    col = m * nchunk + ch
                    if m == 0:
                        # Act: copy + sum accum
                        nc.scalar.activation(
                            out=ybuf[:, sl], in_=ps1[:], func=ACT.Copy,
                            accum_out=s1p[:, col:col + 1])
                    else:
                        # DVE: copy + sum accum in one stt
                        nc.vector.scalar_tensor_tensor(
                            out=ybuf[:, sl], in0=ps1[:], scalar=0.0,
                            in1=ps1[:], op0=ALU.bypass, op1=ALU.bypass,
                            accum_out=s1p[:, col:col + 1])
                    nc.vector.scalar_tensor_tensor(
                        out=scr512[:, 0:csz], in0=ybuf[:, sl], scalar=0.0,
                        in1=ybuf[:, sl], op0=ALU.bypass, op1=ALU.mult,
                        accum_out=q1p[:, col:col + 1])

            # ---------------- layer 2 BN + out ----------------
            g1 = gather_stats(s1p[:], q1p[:], "1", 0, nchunk)
            a1, b1h = bn_stats(g1p, bt1p, "1", [g1])
            for ch in range(nchunk):
                sl = slice(ch * csz, (ch + 1) * csz)
                o = outp.tile([128, csz], F32, tag="outsb")
                nc.scalar.activation(out=o[:], in_=y0a[:, sl],
                                     func=ACT.Relu,
                                     scale=a1[:, 0:1],
                                     bias=b1h[:, 0:1])
                nc.sync.dma_start(d_out[0, :, sl], o[:])
                o2 = outp.tile([128, csz], F32, tag="outsb2")
                nc.vector.tensor_scalar(out=o2[:], in0=y0b[:, sl],
                                        scalar1=a1[:, 1:2],
                                        scalar2=b1h[:, 1:2],
                                        op0=ALU.mult, op1=ALU.add)
                nc.vector.tensor_scalar_max(o2[:], o2[:], 0.0)
                nc.scalar.dma_start(d_out[1, :, sl], o2[:])

    nc.compile()
    return nc


def make_core_inputs(sampled_xyz, sampled_features, original_xyz,
                     original_features, w0, w1, g0, bt0, g1, bt1,
                     core, n_t=QP // 128):
    """Host-side layout prep for one core (transposes/reshapes/casts only)."""
    nq = n_t * 128
    nst = S // 128
    b, h = core // 2, core % 2
    ox = original_xyz[b, h * nq:(h + 1) * nq]          # [nq, 3]
    of = original_features[b, h * nq:(h + 1) * nq]     # [nq, CO]
    sx = sampled_xyz[b]                                # [S, 3]
    f32 = np.float32
    bf16 = ml_dtypes.bfloat16
    xT4 = np.concatenate([ox.T, np.ones((1, nq), f32),
                          np.zeros((28, nq), f32)], 0).astype(f32)
    sn2 = (sx.astype(f32) * sx.astype(f32)).sum(-1).astype(f32)
    rhsS = np.concatenate([(f32(2) * sx.T).astype(f32), -sn2[None, :],
                           np.zeros((28, S), f32)], 0).astype(f32)
    xn2 = (ox.astype(f32) * ox.astype(f32)).sum(-1).astype(f32)
    xn2 = np.ascontiguousarray(xn2.reshape(n_t, 128).T).astype(f32)
    return {
        "xT4": xT4,
        "rhsS": rhsS,
        "xn2": xn2,
        "sfeatT": np.ascontiguousarray(sampled_features[b]).astype(bf16),
        "ofT": np.ascontiguousarray(of.T).astype(bf16),
        "w0T": np.ascontiguousarray(
            w0.T.reshape(3, 128, C1).transpose(1, 0, 2)).astype(bf16),
        "w1T": np.ascontiguousarray(
            w1.T.reshape(2, 128, C2).transpose(1, 0, 2)).astype(bf16),
        "g0p": np.ascontiguousarray(g0.reshape(2, 128).T).astype(f32),
        "bt0p": np.ascontiguousarray(bt0.reshape(2, 128).T).astype(f32),
        "g1p": np.ascontiguousarray(g1.reshape(2, 128).T).astype(f32),
        "bt1p": np.ascontiguousarray(bt1.reshape(2, 128).T).astype(f32),
        "eye": np.eye(128, dtype=f32),
    }


_PROGRAM_CACHE = {}


def kernel(sampled_xyz, sampled_features, original_xyz, original_features,
           w0, b0, g0, bt0, w1, b1, g1, bt1, k):
    assert int(k) == 3
    from concourse.bass_utils import run_bass_kernel_spmd

    key = "full"
    if key not in _PROGRAM_CACHE:
        _PROGRAM_CACHE[key] = build_program()
    nc = _PROGRAM_CACHE[key]

    args = (sampled_xyz, sampled_features, original_xyz, original_features,
            w0, w1, g0, bt0, g1, bt1)
    in_maps = [make_core_inputs(*[np.asarray(a, np.float32) for a in args],
                                core=c) for c in range(NCORES)]
    res = run_bass_kernel_spmd(nc, in_maps, core_ids=list(range(NCORES)))
    out = np.empty((B, N, C2), np.float32)
    nq = QP
    for c in range(NCORES):
        b, h = c // 2, c % 2
        yT = res.results[c]["yT"]            # [2, 128, nq]
        y = yT.reshape(256, nq).T            # [nq, 256]
        out[b, h * nq:(h + 1) * nq] = y
    return out
